# revision 3
# baseline (speedup 1.0000x reference)
"""Trainium2 Bass kernel for the CP-sparse-degree-LU module (fp8 DoubleRow).

Reference computation (all fp32):
    zf  = z.reshape(-1, 2048)                      # [N=8192, d]
    W   = masks * U                                # [6, k, d]
    out = zf @ W[0].T                              # [N, k]
    for i in 1..5: out = (zf @ W[i].T) * out + out
    x   = out @ C_w.T + C_b                        # [N, o]

Sharding: data-parallel over the token dim N across 8 cores (1024 tokens
each), weights replicated; no collectives. Layout is transposed on device
(acc is [k, tok], output is [o, tok]) so the chain and projection run
without transposes.

Precision: every matmul runs in fp8e4 with MatmulPerfMode.DoubleRow (two
128-deep contraction tiles per pass at 0.5 cycles/row — 4x the fp32r
rate). Accuracy is recovered with a 3-term hi/lo decomposition: each
operand X is split host-side into Xh = fp8(s*X) and Xl = fp8(s*X - Xh)
at the SAME scale s, and the product uses
    Xh@Yh + Xh@Yl + Xl@Yh   (the Xl@Yl term is second-order and dropped)
which measures ~2e-3 max-rel error vs the fp32 reference (gate is 2e-2).
Cost is 3 fp8 slots per 128x128 block = 0.75 cycles/row-col vs 1.0 for
fp32r, on top of halved DMA traffic.

Scales: W and C_w are quantized at SW=SC=64 (power of two). The chain
update acc = (mm + 1) * acc becomes acc = (ps + SW) * acc, accumulating
one factor SW per active degree; the per-k-tile cast that produces the
fp8 projection inputs folds SW^-n_active back out. The projection PSUM
is SC * x, folded out in the scalar-engine bias-add activation.

Sparsity: all-zero 128x128 blocks of W are detected host-side and the
device program skips them, exactly as the fp32r baseline did, except
contraction ranges are padded to even 256-wide DoubleRow pairs (a padded
slot is all-zero fp8 and contributes nothing).
"""

import os
import sys
import types
from contextlib import ExitStack

import numpy as np

DEGREE, D, K, O = 6, 2048, 2048, 2048
N_CORES = 8
N_TOTAL = 8192
TOK = N_TOTAL // N_CORES  # 1024 tokens per core
P = 128
DT = D // P  # 16 contraction tiles
KT = K // P  # 16 rank tiles
OT = O // P  # 16 output tiles
NPAIR = DT // 2  # 8 DoubleRow contraction pairs
NC_CHUNK = 512  # moving free dim per matmul (PSUM bank, fp32 max)
TC = TOK // NC_CHUNK  # 2 token chunks
SW = 64.0  # fp8 scale for W
SC = 64.0  # fp8 scale for C_w

_CACHE = {}


def _install_ntff_shim():
    """Register antenv.axon_hooks so run_bass_kernel_spmd(trace=True) can
    profile under axon. Safe no-op if anything is unavailable."""
    try:
        if "antenv.axon_hooks" in sys.modules:
            return
        mod = types.ModuleType("antenv.axon_hooks")
        mod._hook = None
        mod.set_axon_ntff_profile_hook = lambda h: setattr(mod, "_hook", h)
        mod.get_axon_ntff_profile_hook = lambda: mod._hook
        sys.modules["antenv.axon_hooks"] = mod
        from trn_agent_boot.trn_boot import _ntff_profile_via_ctypes

        mod._hook = _ntff_profile_via_ctypes("/opt/axon/libaxon_pjrt.so")
    except Exception:
        pass


def _build(ranges):
    """ranges[i][kt] = (q0, q1) inclusive DoubleRow-pair range, or None if the
    whole (degree, rank-tile) block row is zero."""
    import concourse.tile as tile
    from concourse import bacc, mybir

    f32 = mybir.dt.float32
    f8 = mybir.dt.float8e4
    ADD = mybir.AluOpType.add
    MULT = mybir.AluOpType.mult
    SUB = mybir.AluOpType.subtract
    IDENT = mybir.ActivationFunctionType.Identity
    DR = mybir.MatmulPerfMode.DoubleRow

    nact = [sum(1 for i in range(DEGREE) if ranges[i][kt] is not None)
            for kt in range(KT)]

    nc = bacc.Bacc("TRN2", target_bir_lowering=False, debug=False)

    # z.T hi/lo fp8, tiled: [di, dt*TOK + t] = fp8(z)[t, dt*P + di]
    zh_d = nc.dram_tensor("zh", [P, DT * TOK], f8, kind="ExternalInput")
    zl_d = nc.dram_tensor("zl", [P, DT * TOK], f8, kind="ExternalInput")
    # W hi/lo per degree/rank-tile: [i, kt, di, dt*P + ki] = fp8(SW*W)[i, kt*P+ki, dt*P+di]
    wh_d = nc.dram_tensor("wh", [DEGREE, KT, P, DT * P], f8, kind="ExternalInput")
    wl_d = nc.dram_tensor("wl", [DEGREE, KT, P, DT * P], f8, kind="ExternalInput")
    # C_w hi/lo tiled: [ot, ki, kt*P + oi] = fp8(SC*C_w)[ot*P+oi, kt*P+ki]
    ch_d = nc.dram_tensor("ch", [OT, P, KT * P], f8, kind="ExternalInput")
    cl_d = nc.dram_tensor("cl", [OT, P, KT * P], f8, kind="ExternalInput")
    # C_b tiled: [oi, ot] = C_b[ot*P + oi]
    cb_d = nc.dram_tensor("cb", [P, OT], f32, kind="ExternalInput")
    # x.T: [o, t]
    x_d = nc.dram_tensor("x", [O, TOK], f32, kind="ExternalOutput")

    zh_ap, zl_ap, wh_ap, wl_ap, ch_ap, cl_ap, cb_ap, x_ap = (
        t.ap() for t in (zh_d, zl_d, wh_d, wl_d, ch_d, cl_d, cb_d, x_d)
    )

    with tile.TileContext(nc) as tc, ExitStack() as ctx:
        zpool = ctx.enter_context(tc.tile_pool(name="z", bufs=2 * NPAIR))
        accpool = ctx.enter_context(tc.tile_pool(name="acc", bufs=KT))
        a8pool = ctx.enter_context(tc.tile_pool(name="a8", bufs=2 * NPAIR))
        wpool = ctx.enter_context(tc.tile_pool(name="w", bufs=6))
        cpool = ctx.enter_context(tc.tile_pool(name="c", bufs=4))
        cbpool = ctx.enter_context(tc.tile_pool(name="cb", bufs=1))
        xpool = ctx.enter_context(tc.tile_pool(name="xt", bufs=4))
        pspool = ctx.enter_context(tc.tile_pool(name="ps", bufs=4, space="PSUM"))

        # Resident tiles: z hi/lo per contraction pair (2KB/part each), acc
        # per rank tile (4KB/part), fp8 acc hi/lo per rank pair (2KB/part).
        zh_sb = [zpool.tile([P, 2 * TOK], f8, tag="z", name=f"zh_sb{q}") for q in range(NPAIR)]
        zl_sb = [zpool.tile([P, 2 * TOK], f8, tag="z", name=f"zl_sb{q}") for q in range(NPAIR)]
        acc = [accpool.tile([P, TOK], f32, tag="acc", name=f"acc{j}") for j in range(KT)]
        ah_sb = [a8pool.tile([P, 2 * TOK], f8, tag="a8", name=f"ah_sb{q}") for q in range(NPAIR)]
        al_sb = [a8pool.tile([P, 2 * TOK], f8, tag="a8", name=f"al_sb{q}") for q in range(NPAIR)]
        cb_sb = cbpool.tile([P, OT], f32)

        # Lazy z pair DMAs: with the tril structure of degree 0, z streams in
        # as the early rank-tile groups consume it.
        z_issued = [False] * NPAIR

        def ensure_z(q0_, q1_):
            for q_ in range(q0_, q1_ + 1):
                if not z_issued[q_]:
                    sl_ = slice(2 * q_ * TOK, 2 * (q_ + 1) * TOK)
                    nc.gpsimd.dma_start(zh_sb[q_][:], zh_ap[:, sl_])
                    nc.gpsimd.dma_start(zl_sb[q_][:], zl_ap[:, sl_])
                    z_issued[q_] = True

        def pair3(w2):
            return w2.rearrange("p (s k) -> p s k", s=2)

        # Degree chain over acc[kt-block, tokens].
        for i in range(DEGREE):
            for kt in range(KT):
                rng = ranges[i][kt]
                if rng is None:
                    if i == 0:
                        nc.gpsimd.memset(acc[kt][:], 0.0)
                        if nact[kt] == 0:
                            q8, s8 = kt >> 1, kt & 1
                            sl8 = slice(s8 * TOK, (s8 + 1) * TOK)
                            nc.gpsimd.memset(ah_sb[q8][:, sl8], 0.0)
                            nc.gpsimd.memset(al_sb[q8][:, sl8], 0.0)
                    continue
                q0, q1 = rng
                nq = q1 - q0 + 1
                ensure_z(q0, q1)
                wh_sb = wpool.tile([P, nq * 2 * P], f8, tag="w")
                wl_sb = wpool.tile([P, nq * 2 * P], f8, tag="w")
                csl = slice(2 * q0 * P, 2 * (q1 + 1) * P)
                nc.sync.dma_start(wh_sb[:], wh_ap[i, kt][:, csl])
                nc.sync.dma_start(wl_sb[:], wl_ap[i, kt][:, csl])
                ps = pspool.tile([P, TOK], f32)
                for j in range(nq):
                    q = q0 + j
                    wh_j = pair3(wh_sb[:, j * 2 * P : (j + 1) * 2 * P])
                    wl_j = pair3(wl_sb[:, j * 2 * P : (j + 1) * 2 * P])
                    zh_q = pair3(zh_sb[q][:])
                    zl_q = pair3(zl_sb[q][:])
                    # 3-term hi/lo: Wh@zh + Wh@zl + Wl@zh (Wl@zl dropped).
                    # Wh is lhsT for the first two terms back-to-back.
                    for t, (wt, zt) in enumerate(
                        ((wh_j, zh_q), (wh_j, zl_q), (wl_j, zh_q))
                    ):
                        for tcx in range(TC):
                            sl = slice(tcx * NC_CHUNK, (tcx + 1) * NC_CHUNK)
                            nc.tensor.matmul(
                                ps[:, sl],
                                wt,
                                zt[:, :, sl],
                                start=(j == 0 and t == 0),
                                stop=(j == nq - 1 and t == 2),
                                perf_mode=DR,
                            )
                dst = acc[kt][:]
                if i == 0:
                    nc.vector.tensor_copy(dst, ps[:])
                else:
                    # acc = (mm + SW) * acc — one DVE op; SW factor folded
                    # out in the fp8 cast below.
                    nc.vector.scalar_tensor_tensor(dst, ps[:], SW, dst, ADD, MULT)
                if i == nact_last[kt]:
                    # Final update for this rank tile: emit the fp8 hi/lo
                    # cast feeding the projection. acc = SW^nact * out.
                    q8, s8 = kt >> 1, kt & 1
                    sl8 = slice(s8 * TOK, (s8 + 1) * TOK)
                    sk = float(SW ** -nact[kt])
                    nc.vector.tensor_scalar_mul(ah_sb[q8][:, sl8], dst, sk)
                    nc.vector.scalar_tensor_tensor(
                        al_sb[q8][:, sl8], dst, sk, ah_sb[q8][:, sl8], MULT, SUB
                    )

        # Final projection: x.T[ot-block] = (Ch+Cl) @ (ah+al) / SC + C_b
        nc.sync.dma_start(cb_sb[:], cb_ap)
        for ot in range(OT):
            ch_sb = cpool.tile([P, KT * P], f8, tag="c")
            cl_sb = cpool.tile([P, KT * P], f8, tag="c")
            nc.sync.dma_start(ch_sb[:], ch_ap[ot])
            nc.sync.dma_start(cl_sb[:], cl_ap[ot])
            ps = pspool.tile([P, TOK], f32)
            for q in range(NPAIR):
                ch_q = pair3(ch_sb[:, q * 2 * P : (q + 1) * 2 * P])
                cl_q = pair3(cl_sb[:, q * 2 * P : (q + 1) * 2 * P])
                ah_q = pair3(ah_sb[q][:])
                al_q = pair3(al_sb[q][:])
                for t, (ct, at) in enumerate(
                    ((ch_q, ah_q), (ch_q, al_q), (cl_q, ah_q))
                ):
                    for tcx in range(TC):
                        sl = slice(tcx * NC_CHUNK, (tcx + 1) * NC_CHUNK)
                        nc.tensor.matmul(
                            ps[:, sl],
                            ct,
                            at[:, :, sl],
                            start=(q == 0 and t == 0),
                            stop=(q == NPAIR - 1 and t == 2),
                            perf_mode=DR,
                        )
            xt = xpool.tile([P, TOK], f32)
            nhalf = 2 if ot == OT - 1 else 1
            step = TOK // nhalf
            for h in range(nhalf):
                sl = slice(h * step, (h + 1) * step)
                nc.scalar.activation(
                    xt[:, sl], ps[:, sl], IDENT,
                    bias=cb_sb[:, ot : ot + 1], scale=1.0 / SC,
                )
                nc.gpsimd.dma_start(x_ap[ot * P : (ot + 1) * P, sl], xt[:, sl])

    nc.compile()
    return nc


# nact_last is filled in by kernel() before _build runs (module-level so the
# closure above stays readable): nact_last[kt] = last active degree for kt.
nact_last = [0] * KT


def kernel(z, U, masks, C_w, C_b):
    import ml_dtypes
    from concourse.bass_utils import run_bass_kernel_spmd

    if os.environ.get("BASS_TRACE"):
        _install_ntff_shim()

    E4 = ml_dtypes.float8_e4m3

    lead = z.shape[:-1]
    zf = np.ascontiguousarray(np.asarray(z, dtype=np.float32).reshape(-1, D))
    W = np.asarray(masks, dtype=np.float32) * np.asarray(U, dtype=np.float32)
    C_w = np.asarray(C_w, dtype=np.float32)
    C_b = np.asarray(C_b, dtype=np.float32)

    # Detect all-zero 128x128 blocks of W; build per-(degree, rank-tile)
    # contraction ranges in even-aligned DoubleRow pairs. Only provably-zero
    # blocks are skipped; pad slots are zero-filled fp8.
    blk = (
        np.abs(W.reshape(DEGREE, KT, P, DT, P)).max(axis=(2, 4)) > 0.0
    )  # [i, kt, dt]
    ranges = []
    for i in range(DEGREE):
        row = []
        for kt in range(KT):
            nz = np.flatnonzero(blk[i, kt])
            row.append((int(nz[0]) >> 1, int(nz[-1]) >> 1) if len(nz) else None)
        ranges.append(tuple(row))
    ranges = tuple(ranges)
    for kt in range(KT):
        act = [i for i in range(DEGREE) if ranges[i][kt] is not None]
        nact_last[kt] = act[-1] if act else 0

    # Host-side hi/lo fp8 split at a shared power-of-two scale.
    def split8(x, s):
        hi = (s * x).astype(E4)
        lo = (s * x - hi.astype(np.float32)).astype(E4)
        return hi, lo

    zh, zl = split8(zf, 1.0)
    Wh, Wl = split8(W, SW)
    Ch, Cl = split8(C_w, SC)

    def w_dev(a):  # [i, kt, di, dt*P + ki]
        return np.ascontiguousarray(
            a.reshape(DEGREE, KT, P, DT, P).transpose(0, 1, 4, 3, 2)
        ).reshape(DEGREE, KT, P, DT * P)

    def c_dev(a):  # [ot, ki, kt*P + oi]
        return np.ascontiguousarray(
            a.reshape(OT, P, KT, P).transpose(0, 3, 2, 1)
        ).reshape(OT, P, KT * P)

    def z_dev(a):  # [di, dt*TOK + t] per core
        return np.ascontiguousarray(
            a.T.reshape(DT, P, TOK).transpose(1, 0, 2)
        ).reshape(P, DT * TOK)

    wh_h, wl_h = w_dev(Wh), w_dev(Wl)
    ch_h, cl_h = c_dev(Ch), c_dev(Cl)
    cb_dev = np.ascontiguousarray(C_b.reshape(OT, P).T)

    in_maps = []
    for c in range(N_CORES):
        ts = slice(c * TOK, (c + 1) * TOK)
        in_maps.append(
            {
                "zh": z_dev(zh[ts]),
                "zl": z_dev(zl[ts]),
                "wh": wh_h,
                "wl": wl_h,
                "ch": ch_h,
                "cl": cl_h,
                "cb": cb_dev,
            }
        )

    if _CACHE.get("ranges") != ranges:
        _CACHE["nc"] = _build(ranges)
        _CACHE["ranges"] = ranges
    nc = _CACHE["nc"]

    res = run_bass_kernel_spmd(nc, in_maps, core_ids=list(range(N_CORES)))
    _CACHE["last_result"] = res

    parts = [res.results[c]["x"].T for c in range(N_CORES)]  # each [TOK, O]
    x = np.concatenate(parts, axis=0)
    return x.reshape(*lead, O)


# revision 4
# speedup vs baseline: 1.5334x; 1.5334x over previous
"""Trainium2 Bass kernel for the CP-sparse-degree-LU module (bf16 matmuls).

Reference computation (all fp32):
    zf  = z.reshape(-1, 2048)                      # [N=8192, d]
    W   = masks * U                                # [6, k, d]
    out = zf @ W[0].T                              # [N, k]
    for i in 1..5: out = (zf @ W[i].T) * out + out
    x   = out @ C_w.T + C_b                        # [N, o]

Sharding: data-parallel over the token dim N across 8 cores (1024 tokens
each), weights replicated; no collectives. Everything is laid out
transposed on device (acc is [k, tok], output is [o, tok]) so the degree
chain and the final projection both run without on-device transposes:
    acc.T = W_i @ z.T  -> lhsT = W_i.T tiles [d,k], rhs = z.T [d, tok]
    x.T   = C_w @ acc  -> lhsT = C_w.T tiles [k,o], rhs = acc [k, tok]

Sparsity: W = masks*U is block-sparse (tril/triu factors plus a degree
mask that zeroes rank rows < i*K/DEGREE at degree i). The host detects
all-zero 128x128 blocks of the actual W at runtime and builds the device
program skipping them — only provably-zero blocks are skipped, so dense
inputs yield the dense program.

Precision: all matmul operands are bf16 (quantized host-side), which
streams at the same 1 output column/cycle as fp32r but halves weight
loads (fully hidden under the matmul), DMA traffic, and SBUF footprint.
PSUM accumulation stays fp32; acc is stored bf16 (it is the moving
operand of the projection). Measured end-to-end error ~6e-3 max-rel vs
the fp32 reference (gate 2e-2).
"""

import os
import sys
import types
from contextlib import ExitStack

import numpy as np

DEGREE, D, K, O = 6, 2048, 2048, 2048
N_CORES = 8
N_TOTAL = 8192
TOK = N_TOTAL // N_CORES  # 1024 tokens per core
P = 128
DT = D // P  # 16 contraction tiles (degree matmuls)
KT = K // P  # 16 rank tiles
OT = O // P  # 16 output tiles
NC_CHUNK = 512  # moving free dim per matmul (PSUM bank, fp32 max)
TC = TOK // NC_CHUNK  # 2 token chunks

_CACHE = {}


def _install_ntff_shim():
    """Register antenv.axon_hooks so run_bass_kernel_spmd(trace=True) can
    profile under axon. Safe no-op if anything is unavailable."""
    try:
        if "antenv.axon_hooks" in sys.modules:
            return
        mod = types.ModuleType("antenv.axon_hooks")
        mod._hook = None
        mod.set_axon_ntff_profile_hook = lambda h: setattr(mod, "_hook", h)
        mod.get_axon_ntff_profile_hook = lambda: mod._hook
        sys.modules["antenv.axon_hooks"] = mod
        from trn_agent_boot.trn_boot import _ntff_profile_via_ctypes

        mod._hook = _ntff_profile_via_ctypes("/opt/axon/libaxon_pjrt.so")
    except Exception:
        pass


def _build(ranges):
    """ranges[i][kt] = (dt_lo, dt_hi) inclusive active range, or None if the
    whole (degree, rank-tile) block row is zero."""
    import concourse.tile as tile
    from concourse import bacc, mybir

    f32 = mybir.dt.float32
    bf16 = mybir.dt.bfloat16
    ADD = mybir.AluOpType.add
    MULT = mybir.AluOpType.mult

    nc = bacc.Bacc("TRN2", target_bir_lowering=False, debug=False)

    # z.T per core, tiled: [di, dt*TOK + t] = z[t, dt*P + di]
    z_d = nc.dram_tensor("z", [P, DT * TOK], bf16, kind="ExternalInput")
    # W per degree/rank-tile: [i, kt, di, dt*P + ki] = W[i, kt*P+ki, dt*P+di]
    w_d = nc.dram_tensor("w", [DEGREE, KT, P, DT * P], bf16, kind="ExternalInput")
    # C_w tiled: [ot, ki, kt*P + oi] = C_w[ot*P+oi, kt*P+ki]
    c_d = nc.dram_tensor("c", [OT, P, KT * P], bf16, kind="ExternalInput")
    # C_b tiled: [oi, ot] = C_b[ot*P + oi]
    cb_d = nc.dram_tensor("cb", [P, OT], f32, kind="ExternalInput")
    # x.T: [o, t]
    x_d = nc.dram_tensor("x", [O, TOK], f32, kind="ExternalOutput")

    z_ap, w_ap, c_ap, cb_ap, x_ap = (t.ap() for t in (z_d, w_d, c_d, cb_d, x_d))

    with tile.TileContext(nc) as tc, ExitStack() as ctx:
        zpool = ctx.enter_context(tc.tile_pool(name="z", bufs=DT))
        accpool = ctx.enter_context(tc.tile_pool(name="acc", bufs=KT))
        wpool = ctx.enter_context(tc.tile_pool(name="w", bufs=5))
        cbpool = ctx.enter_context(tc.tile_pool(name="cb", bufs=1))
        xpool = ctx.enter_context(tc.tile_pool(name="xt", bufs=4))
        pspool = ctx.enter_context(tc.tile_pool(name="ps", bufs=4, space="PSUM"))

        # Resident per-tile buffers: z.T (16x2KB/part) and acc (16x2KB/part).
        # Separate tiles give the scheduler fine-grained deps — readers of
        # z[dt] start as soon as that slice's DMA lands.
        z_sb = [zpool.tile([P, TOK], bf16, tag="z", name=f"z_sb{j}") for j in range(DT)]
        acc = [accpool.tile([P, TOK], bf16, tag="acc", name=f"acc{j}") for j in range(KT)]
        cb_sb = cbpool.tile([P, OT], f32)

        # Issue each z[dt] DMA lazily, right before the first group that
        # reads it — with the tril structure of degree 0 this streams z in
        # as the early rank-tile groups consume it instead of serializing
        # the whole load ahead of the first weight tile.
        z_issued = [False] * DT

        def ensure_z(lo_, hi_):
            for dt_ in range(lo_, hi_ + 1):
                if not z_issued[dt_]:
                    nc.gpsimd.dma_start(
                        z_sb[dt_][:], z_ap[:, dt_ * TOK : (dt_ + 1) * TOK]
                    )
                    z_issued[dt_] = True

        # Degree chain over acc[kt-block, tokens].
        for i in range(DEGREE):
            for kt in range(KT):
                rng = ranges[i][kt]
                if rng is None:
                    if i == 0:
                        # acc = mm = 0 for this rank block
                        nc.gpsimd.memset(acc[kt][:], 0.0)
                    continue
                lo, hi = rng
                ndt = hi - lo + 1
                ensure_z(lo, hi)
                w_sb = wpool.tile([P, ndt * P], bf16, tag="w")
                nc.sync.dma_start(w_sb[:], w_ap[i, kt][:, lo * P : (hi + 1) * P])
                ps = pspool.tile([P, TOK], f32)
                for tcx in range(TC):
                    for j, dt in enumerate(range(lo, hi + 1)):
                        nc.tensor.matmul(
                            ps[:, tcx * NC_CHUNK : (tcx + 1) * NC_CHUNK],
                            w_sb[:, j * P : (j + 1) * P],
                            z_sb[dt][:, tcx * NC_CHUNK : (tcx + 1) * NC_CHUNK],
                            start=(j == 0),
                            stop=(j == ndt - 1),
                        )
                dst = acc[kt][:]
                if i == 0:
                    nc.vector.tensor_copy(dst, ps[:])
                else:
                    # acc = (mm + 1) * acc  — one DVE op
                    nc.vector.scalar_tensor_tensor(dst, ps[:], 1.0, dst, ADD, MULT)

        # Final projection: x.T[ot-block] = C_w @ acc + C_b
        nc.sync.dma_start(cb_sb[:], cb_ap)
        for ot in range(OT):
            c_sb = wpool.tile([P, KT * P], bf16, tag="w")
            nc.sync.dma_start(c_sb[:], c_ap[ot])
            ps = pspool.tile([P, TOK], f32)
            for tcx in range(TC):
                for kt in range(KT):
                    nc.tensor.matmul(
                        ps[:, tcx * NC_CHUNK : (tcx + 1) * NC_CHUNK],
                        c_sb[:, kt * P : (kt + 1) * P],
                        acc[kt][:, tcx * NC_CHUNK : (tcx + 1) * NC_CHUNK],
                        start=(kt == 0),
                        stop=(kt == KT - 1),
                    )
            xt = xpool.tile([P, TOK], f32)
            nhalf = 2 if ot == OT - 1 else 1
            step = TOK // nhalf
            for h in range(nhalf):
                sl = slice(h * step, (h + 1) * step)
                nc.vector.tensor_scalar_add(xt[:, sl], ps[:, sl], cb_sb[:, ot : ot + 1])
                nc.gpsimd.dma_start(x_ap[ot * P : (ot + 1) * P, sl], xt[:, sl])

    nc.compile()
    return nc


def kernel(z, U, masks, C_w, C_b):
    import ml_dtypes
    from concourse.bass_utils import run_bass_kernel_spmd

    if os.environ.get("BASS_TRACE"):
        _install_ntff_shim()

    BF16 = ml_dtypes.bfloat16

    lead = z.shape[:-1]
    zf = np.ascontiguousarray(np.asarray(z, dtype=np.float32).reshape(-1, D))
    W = np.asarray(masks, dtype=np.float32) * np.asarray(U, dtype=np.float32)
    C_w = np.asarray(C_w, dtype=np.float32)
    C_b = np.asarray(C_b, dtype=np.float32)

    # Detect all-zero 128x128 blocks of W; build per-(degree, rank-tile)
    # contraction ranges. Only provably-zero blocks are skipped.
    blk = (
        np.abs(W.reshape(DEGREE, KT, P, DT, P)).max(axis=(2, 4)) > 0.0
    )  # [i, kt, dt]
    ranges = []
    for i in range(DEGREE):
        row = []
        for kt in range(KT):
            nz = np.flatnonzero(blk[i, kt])
            row.append((int(nz[0]), int(nz[-1])) if len(nz) else None)
        ranges.append(tuple(row))
    ranges = tuple(ranges)

    # Device layouts (see _build for index conventions), quantized to bf16.
    w_dev = np.ascontiguousarray(
        W.astype(BF16).reshape(DEGREE, KT, P, DT, P).transpose(0, 1, 4, 3, 2)
    ).reshape(DEGREE, KT, P, DT * P)
    c_dev = np.ascontiguousarray(
        C_w.astype(BF16).reshape(OT, P, KT, P).transpose(0, 3, 2, 1)
    ).reshape(OT, P, KT * P)
    cb_dev = np.ascontiguousarray(C_b.reshape(OT, P).T)
    zb = zf.astype(BF16)

    in_maps = []
    for c in range(N_CORES):
        zs = zb[c * TOK : (c + 1) * TOK]  # [TOK, D]
        z_dev = np.ascontiguousarray(
            zs.T.reshape(DT, P, TOK).transpose(1, 0, 2)
        ).reshape(P, DT * TOK)
        in_maps.append({"z": z_dev, "w": w_dev, "c": c_dev, "cb": cb_dev})

    if _CACHE.get("ranges") != ranges:
        _CACHE["nc"] = _build(ranges)
        _CACHE["ranges"] = ranges
    nc = _CACHE["nc"]

    res = run_bass_kernel_spmd(nc, in_maps, core_ids=list(range(N_CORES)))
    _CACHE["last_result"] = res

    parts = [res.results[c]["x"].T for c in range(N_CORES)]  # each [TOK, O]
    x = np.concatenate(parts, axis=0)
    return x.reshape(*lead, O)
